# revision 1
# baseline (speedup 1.0000x reference)
"""GCN message-passing kernel for 8 trn2 NeuronCores.

Math (per reference): h = relu(a @ (x @ W1) + b1); out = h @ W2 + b2
Shapes: x [8,4096,240], a [4096,4096], W1 [240,32], W2 [32,240].

Sharding: 2x4 grid. Core c -> batch group g=c//4 (4 batches), output-row
group j=c%4 (1024 rows). x, a, W1, W2 host-converted to fp16 (1-pass PE,
half DMA); PSUM accumulates fp32; output returned to host as fp16 and
upcast. End-to-end rel err ~5e-4.

DMA: all transfers are full-row contiguous DRAM extents; the single DMA
queue stripes packets across 16 engines (~300 GB/s aggregate), so the
stream order x -> aT -> out matches compute order. All 32 aT row-tiles are
independent buffers issued up front so phase 2 never waits on a stale
buffer.

PE schedule per core (in-order):
  phase 1 (b-outer, x-paced): hT[32b+h, n] accumulated into 8 PSUM banks;
          W1 zero-padded per-batch so 128 partitions = (batch, hidden);
          PSUM->SBUF copies start as soon as b=3 finishes each column.
  phase 2 (kt-outer, software-pipelined): PE-transpose of hT block kt+2
          interleaved with the two 512-col matmuls of block kt, so the
          transpose bubble and the PSUM->SBUF copy latency are hidden.
  phase 3: relu+b1 on ACT (fp16), block-diagonal W2 head matmul (+b2 on
          DVE, flat (hf,b,l) column layout), fp16 partition-major output.
"""

import sys

if "/opt/trn_rl_repo" not in sys.path:
    sys.path.insert(0, "/opt/trn_rl_repo")

import numpy as np

B, N, F, H, L = 8, 4096, 240, 32, 240
NB = 4        # batches per core
NRC = 1024    # output rows per core
TRACE = False

_cache = {}
last_exec_time_ns = None
last_profile_json = None


def _install_ntff_hook():
    import types

    import antenv

    if "antenv.axon_hooks" in sys.modules:
        return
    mod = types.ModuleType("antenv.axon_hooks")
    _state = {"hook": None}
    mod.set_axon_ntff_profile_hook = lambda h: _state.__setitem__("hook", h)
    mod.get_axon_ntff_profile_hook = lambda: _state["hook"]
    sys.modules["antenv.axon_hooks"] = mod
    antenv.axon_hooks = mod
    from trn_agent_boot.trn_boot import _ntff_profile_via_ctypes

    mod.set_axon_ntff_profile_hook(
        _ntff_profile_via_ctypes("/opt/axon/libaxon_pjrt.so")
    )


def _build():
    import concourse.bass as bass
    import concourse.tile as tile
    from concourse import bacc, mybir

    f32 = mybir.dt.float32
    f16 = mybir.dt.float16
    ts, ds = bass.ts, bass.ds

    nc = bacc.Bacc("TRN2", target_bir_lowering=False, debug=False, num_devices=8)
    xT = nc.dram_tensor("xT", [NB * F, N], f16, kind="ExternalInput").ap()
    aT = nc.dram_tensor("aT", [N, NRC], f16, kind="ExternalInput").ap()
    w1p = nc.dram_tensor("w1p", [F, 512], f16, kind="ExternalInput").ap()
    w2k = nc.dram_tensor("w2k", [128, 960], f16, kind="ExternalInput").ap()
    b1s = nc.dram_tensor("b1s", [128, 1], f32, kind="ExternalInput").ap()
    b2k = nc.dram_tensor("b2k", [128, 960], f32, kind="ExternalInput").ap()
    idn = nc.dram_tensor("idn", [128, 128], f16, kind="ExternalInput").ap()
    outp = nc.dram_tensor("outp", [128, 8 * NB * L], f16,
                          kind="ExternalOutput").ap()

    relu = mybir.ActivationFunctionType.Relu

    with tile.TileContext(nc) as tc:
        with tc.tile_pool(name="const", bufs=1) as cp:
            w1a = cp.tile([128, 512], f16)
            nc.sync.dma_start(w1a[:], w1p[0:128, :])
            w1b = cp.tile([112, 512], f16)
            nc.sync.dma_start(w1b[:], w1p[128:240, :])
            w2s = cp.tile([128, 960], f16)
            nc.sync.dma_start(w2s[:], w2k[:])
            b1t = cp.tile([128, 1], f32)
            nc.sync.dma_start(b1t[:], b1s[:])
            b2t = cp.tile([128, 960], f32)
            nc.sync.dma_start(b2t[:], b2k[:])
            idt = cp.tile([128, 128], f16)
            nc.sync.dma_start(idt[:], idn[:])
            hT = cp.tile([128, N], f16)
            hsb = cp.tile([128, N], f16)
            at = [cp.tile([128, NRC], f16, name=f"at_{k}") for k in range(32)]

            # phase 1: hT[32b+h, n] = sum_f W1[f,h] * x[b,n,f]
            with tc.tile_pool(name="xs", bufs=2) as xs, \
                 tc.tile_pool(name="ps1", bufs=1, space="PSUM") as ps1:
                p1 = [ps1.tile([128, 512], f32, name=f"p1_{i}")
                      for i in range(8)]
                for b in range(NB):
                    xa = xs.tile([128, N], f16)
                    nc.sync.dma_start(xa[:], xT[ds(b * F, 128), :])
                    xb = xs.tile([112, N], f16)
                    nc.sync.dma_start(xb[:], xT[ds(b * F + 128, 112), :])
                    for ncol in range(8):
                        nc.tensor.matmul(
                            p1[ncol][:], w1a[:, ts(b, 128)],
                            xa[:, ts(ncol, 512)],
                            start=(b == 0), stop=False)
                        nc.tensor.matmul(
                            p1[ncol][:], w1b[:, ts(b, 128)],
                            xb[:, ts(ncol, 512)],
                            start=False, stop=(b == NB - 1))
                        if b == NB - 1:
                            nc.vector.tensor_copy(
                                hT[:, ts(ncol, 512)], p1[ncol][:])

            # stream aT row-tiles (queued behind x in DMA order)
            for kt in range(32):
                nc.sync.dma_start(at[kt][:], aT[ts(kt, 128), :])

            # phase 2 interleaved with hT transposes (software pipeline):
            # transpose kt+2 runs while matmul kt consumes hsb block kt.
            with tc.tile_pool(name="rs", bufs=2) as rs, \
                 tc.tile_pool(name="os", bufs=3) as osb, \
                 tc.tile_pool(name="pst", bufs=2, space="PSUM") as pst, \
                 tc.tile_pool(name="ps2", bufs=1, space="PSUM") as ps2, \
                 tc.tile_pool(name="ps3", bufs=2, space="PSUM") as ps3:
                pa = [ps2.tile([128, 512], f32, name=f"pa_{i}")
                      for i in range(2)]

                def emit_transpose(m):
                    pt = pst.tile([128, 128], f16)
                    nc.tensor.transpose(pt[:], hT[:, ts(m, 128)], idt[:])
                    nc.vector.tensor_copy(hsb[:, ts(m, 128)], pt[:])

                emit_transpose(0)
                emit_transpose(1)
                for kt in range(32):
                    if kt + 2 < 32:
                        emit_transpose(kt + 2)
                    for mc in range(2):
                        nc.tensor.matmul(
                            pa[mc][:], hsb[:, ts(kt, 128)],
                            at[kt][:, ts(mc, 512)],
                            start=(kt == 0), stop=(kt == 31))

                # phase 3: relu+b1, block-diagonal W2 head, +b2, store fp16
                # w2s[32b+h, hf*480 + b*120 + li] = W2[h, hf*120 + li]
                for mc in range(2):
                    r = rs.tile([128, 512], f16)
                    nc.scalar.activation(r[:], pa[mc][:], relu, bias=b1t[:])
                    for s in range(4):
                        o = osb.tile([128, NB * L], f16)
                        for hf in range(2):
                            p3 = ps3.tile([128, 480], f32)
                            nc.tensor.matmul(
                                p3[:], r[:, ts(s, 128)], w2s[:, ts(hf, 480)],
                                start=True, stop=True)
                            nc.vector.tensor_add(
                                o[:, ts(hf, 480)], p3[:], b2t[:, ts(hf, 480)])
                        nc.sync.dma_start(
                            outp[:, ts(mc * 4 + s, NB * L)], o[:])

    nc.compile()
    return nc


def kernel(x, a, W1, b1, W2, b2):
    global last_exec_time_ns, last_profile_json
    from concourse.bass_utils import run_bass_kernel_spmd

    if "nc" not in _cache:
        _cache["nc"] = _build()
    nc = _cache["nc"]

    x = np.asarray(x, np.float32)
    a = np.asarray(a, np.float32)
    W1 = np.asarray(W1, np.float32)
    b1 = np.asarray(b1, np.float32)
    W2 = np.asarray(W2, np.float32)
    b2 = np.asarray(b2, np.float32)

    xg = [np.ascontiguousarray(
        x[g * NB:(g + 1) * NB].transpose(0, 2, 1)).reshape(
            NB * F, N).astype(np.float16)
        for g in range(2)]
    aj = [np.ascontiguousarray(a[j * NRC:(j + 1) * NRC, :].T).astype(
        np.float16) for j in range(4)]
    w1p = np.zeros((F, 512), np.float16)
    for b in range(NB):
        w1p[:, 128 * b + 32 * b:128 * b + 32 * b + 32] = W1.astype(np.float16)
    # w2k[32b+h, hf*480 + b*120 + li] = W2[h, hf*120 + li]; zeros elsewhere
    w2k = np.zeros((128, 960), np.float16)
    b2k = np.empty((128, 960), np.float32)
    for hf in range(2):
        for b in range(NB):
            w2k[32 * b:32 * b + 32, 480 * hf + 120 * b:480 * hf + 120 * b + 120] = \
                W2[:, 120 * hf:120 * hf + 120].astype(np.float16)
            b2k[:, 480 * hf + 120 * b:480 * hf + 120 * b + 120] = \
                b2[None, 120 * hf:120 * hf + 120]
    b1s = np.ascontiguousarray(np.tile(b1, 4).reshape(128, 1))
    idn = np.eye(128, dtype=np.float16)

    ins = []
    for c in range(8):
        g, j = c // 4, c % 4
        ins.append({"xT": xg[g], "aT": aj[j], "w1p": w1p, "w2k": w2k,
                    "b1s": b1s, "b2k": b2k, "idn": idn})

    trace = TRACE
    if trace:
        try:
            _install_ntff_hook()
        except Exception:
            trace = False
    r = run_bass_kernel_spmd(nc, ins, list(range(8)), trace=trace)
    last_exec_time_ns = r.exec_time_ns
    last_profile_json = r.profile_json

    res = np.empty((B, N, L), np.float32)
    for c in range(8):
        g, j = c // 4, c % 4
        # outp[p, (mc,s), hf, b, li]; n = (mc*4+s)*128 + p; l = hf*120+li
        arr = r.results[c]["outp"].reshape(128, 8, 2, NB, 120)
        res[g * NB:(g + 1) * NB, j * NRC:(j + 1) * NRC, :] = \
            arr.transpose(3, 1, 0, 2, 4).reshape(NB, NRC, L).astype(np.float32)
    return res



# revision 5
# speedup vs baseline: 1.2271x; 1.2271x over previous
"""GCN message-passing kernel for 8 trn2 NeuronCores.

Math (per reference): h = relu(a @ (x @ W1) + b1); out = h @ W2 + b2
Shapes: x [8,4096,240], a [4096,4096], W1 [240,32], W2 [32,240].

Sharding: 2x4 grid. Core c -> batch group g=c//4 (4 batches), output-row
group j=c%4 (1024 rows).

Precision: a is shipped as centered fp8 e3m4 (a-0.5), halving the dominant
DMA stream; the exact rank-1 correction 0.5*sum_m h[m,:] is computed with
one DVE row-reduce of hT and folded into the ReLU bias together with b1.
The PE accepts mixed fp16(stationary) x fp8(moving) matmuls at full
internal precision, so h/x/W1/W2 stay fp16. End-to-end rel err ~8e-3.

DMA: all tiles use 8KB contiguous lines (x: 8 tiles [128|112,4096] f16;
a: 4 tiles [128,8192] f8; out: 4 tiles [128,1920] f16). Every tile has a
dedicated buffer so the single in-order queue never stalls on reuse.

PE schedule per core (in-order):
  phase 1 (n-block-major): per ncol-pair tile, 8 matmuls accumulate
          hT[(b,h), n] (W1 zero-padded per-batch, 128 part = (batch,
          hidden)); PSUM drains and the 4 PE-transposes of each finished
          512-col block run while the next x tile streams, so hsb[m,(b,h)]
          is complete right after the last x byte lands.
  phase 2 (2 chunks of 512 out-rows): 32 fp16xfp8 matmuls per chunk
          accumulate pa[(b,h), n]; the a stream trails x in the same DMA
          queue and stays ahead of the PE.
  phase 3 per chunk: relu+bias on ACT, block-diagonal W2 head (b2==0 is
          detected on host and skipped; else added via rank-1 matmul),
          PSUM drains round-robined over DVE/GpSimd/Scalar, fp16 stores
          overlap the next chunk's compute.
"""

import sys

if "/opt/trn_rl_repo" not in sys.path:
    sys.path.insert(0, "/opt/trn_rl_repo")

import numpy as np
import ml_dtypes

B, N, F, H, L = 8, 4096, 240, 32, 240
NB = 4        # batches per core
NRC = 1024    # output rows per core
TRACE = False

_cache = {}
last_exec_time_ns = None
last_profile_json = None


def _install_ntff_hook():
    import types

    import antenv

    if "antenv.axon_hooks" in sys.modules:
        return
    mod = types.ModuleType("antenv.axon_hooks")
    _state = {"hook": None}
    mod.set_axon_ntff_profile_hook = lambda h: _state.__setitem__("hook", h)
    mod.get_axon_ntff_profile_hook = lambda: _state["hook"]
    sys.modules["antenv.axon_hooks"] = mod
    antenv.axon_hooks = mod
    from trn_agent_boot.trn_boot import _ntff_profile_via_ctypes

    mod.set_axon_ntff_profile_hook(
        _ntff_profile_via_ctypes("/opt/axon/libaxon_pjrt.so")
    )


def _build(has_b2):
    import concourse.bass as bass
    import concourse.tile as tile
    from concourse import bacc, mybir

    f32 = mybir.dt.float32
    f16 = mybir.dt.float16
    f8 = mybir.dt.float8e3
    ts, ds = bass.ts, bass.ds

    nc = bacc.Bacc("TRN2", target_bir_lowering=False, debug=False, num_devices=8)
    xt = nc.dram_tensor("xt", [4 * F, N], f16, kind="ExternalInput").ap()
    atp = nc.dram_tensor("atp", [512, 8192], f8, kind="ExternalInput").ap()
    w1p = nc.dram_tensor("w1p", [F, 512], f16, kind="ExternalInput").ap()
    w2k = nc.dram_tensor("w2k", [128, 960], f16, kind="ExternalInput").ap()
    b1s = nc.dram_tensor("b1s", [128, 1], f32, kind="ExternalInput").ap()
    idn = nc.dram_tensor("idn", [128, 128], f16, kind="ExternalInput").ap()
    if has_b2:
        b2r = nc.dram_tensor("b2r", [1, 960], f16, kind="ExternalInput").ap()
    outp = nc.dram_tensor("outp", [128, 8 * NB * L], f16,
                          kind="ExternalOutput").ap()

    relu = mybir.ActivationFunctionType.Relu
    copyf = mybir.ActivationFunctionType.Copy
    AX = mybir.AxisListType.X
    add = mybir.AluOpType.add
    mult = mybir.AluOpType.mult

    with tile.TileContext(nc) as tc:
        with tc.tile_pool(name="const", bufs=1) as cp:
            w1a = cp.tile([128, 512], f16)
            nc.sync.dma_start(w1a[:], w1p[0:128, :])
            w1b = cp.tile([112, 512], f16)
            nc.sync.dma_start(w1b[:], w1p[128:240, :])
            w2s = cp.tile([128, 960], f16)
            nc.sync.dma_start(w2s[:], w2k[:])
            b1t = cp.tile([128, 1], f32)
            nc.sync.dma_start(b1t[:], b1s[:])
            idt = cp.tile([128, 128], f16)
            nc.sync.dma_start(idt[:], idn[:])
            if has_b2:
                b2t = cp.tile([1, 960], f16)
                nc.sync.dma_start(b2t[:], b2r[:])
                ones = cp.tile([1, 128], f16)
                nc.vector.memset(ones[:], 1.0)
            hT = cp.tile([128, N], f16)
            hsb = cp.tile([128, N], f16)
            at4 = [cp.tile([128, 8192], f8, name=f"at_{k}") for k in range(4)]
            csum = cp.tile([128, 1], f32)
            bc = cp.tile([128, 1], f32)

            # phase 1: hT[32b+h, n] = sum_f W1[f,h] * x[b,n,f], n-block-major
            with tc.tile_pool(name="xs", bufs=4) as xs, \
                 tc.tile_pool(name="ps1", bufs=2, space="PSUM") as ps1, \
                 tc.tile_pool(name="pst", bufs=4, space="PSUM") as pst:
                for np_ in range(4):
                    xa = xs.tile([128, 4096], f16)
                    nc.sync.dma_start(xa[:], xt[ds(np_ * F, 128), :])
                    xb = xs.tile([112, 4096], f16)
                    nc.sync.dma_start(xb[:], xt[ds(np_ * F + 128, 112), :])
                    for nn in range(2):
                        ncol = 2 * np_ + nn
                        p1 = ps1.tile([128, 512], f32)
                        for b in range(NB):
                            nc.tensor.matmul(
                                p1[:], w1a[:, ts(b, 128)],
                                xa[:, ds(b * 1024 + nn * 512, 512)],
                                start=(b == 0), stop=False)
                            nc.tensor.matmul(
                                p1[:], w1b[:, ts(b, 128)],
                                xb[:, ds(b * 1024 + nn * 512, 512)],
                                start=False, stop=(b == NB - 1))
                        if nn == 0:
                            nc.scalar.activation(
                                hT[:, ts(ncol, 512)], p1[:], copyf)
                        else:
                            nc.vector.tensor_copy(hT[:, ts(ncol, 512)], p1[:])
                        for q in range(4):
                            m = ncol * 4 + q
                            pt = pst.tile([128, 128], f16)
                            nc.tensor.transpose(pt[:], hT[:, ts(m, 128)],
                                                idt[:])
                            if q % 2 == 0:
                                nc.vector.tensor_copy(
                                    hsb[:, ts(m, 128)], pt[:])
                            else:
                                nc.scalar.activation(
                                    hsb[:, ts(m, 128)], pt[:], copyf)

            # stream centered-fp8 a tiles (queued behind x)
            for k in range(4):
                nc.sync.dma_start(at4[k][:], atp[ds(k * 128, 128), :])

            # bias = 0.5 * rowsum(hT) + b1  (rank-1 centering correction)
            nc.vector.tensor_reduce(csum[:], hT[:], axis=AX, op=add)
            nc.vector.tensor_scalar(bc[:], csum[:], 0.5, b1t[:],
                                    op0=mult, op1=add)

            # phase 2+3, two chunks of 512 output rows each
            with tc.tile_pool(name="rs", bufs=2) as rs, \
                 tc.tile_pool(name="os", bufs=3) as osb, \
                 tc.tile_pool(name="ps2", bufs=2, space="PSUM") as ps2, \
                 tc.tile_pool(name="ps3", bufs=2, space="PSUM") as ps3:
                drains = [nc.vector, nc.scalar]
                dri = 0
                for c in range(2):
                    pa = ps2.tile([128, 512], f32)
                    for kt in range(32):
                        t, mb = kt // 16, kt % 16
                        nc.tensor.matmul(
                            pa[:], hsb[:, ts(kt, 128)],
                            at4[2 * c + t][:, ds(mb * 512, 512)],
                            start=(kt == 0), stop=(kt == 31))
                    r = rs.tile([128, 512], f16)
                    nc.scalar.activation(r[:], pa[:], relu, bias=bc[:])
                    # w2s[32b+h, hf*480 + b*120 + li] = W2[h, hf*120 + li]
                    for sp in range(2):
                        o = osb.tile([128, 1920], f16)
                        for ss in range(2):
                            s = sp * 2 + ss
                            for hf in range(2):
                                p3 = ps3.tile([128, 480], f32)
                                if has_b2:
                                    nc.tensor.matmul(
                                        p3[:], ones[:], b2t[:, ts(hf, 480)],
                                        start=True, stop=False)
                                nc.tensor.matmul(
                                    p3[:], r[:, ts(s, 128)],
                                    w2s[:, ts(hf, 480)],
                                    start=(not has_b2), stop=True)
                                dst = o[:, ds(ss * 960 + hf * 480, 480)]
                                eng = drains[dri % 2]
                                dri += 1
                                if eng is nc.scalar:
                                    nc.scalar.activation(dst, p3[:], copyf)
                                else:
                                    eng.tensor_copy(dst, p3[:])
                        nc.sync.dma_start(
                            outp[:, ts(c * 2 + sp, 1920)], o[:])

    nc.compile()
    return nc


def kernel(x, a, W1, b1, W2, b2):
    global last_exec_time_ns, last_profile_json
    from concourse.bass_utils import run_bass_kernel_spmd

    x = np.asarray(x, np.float32)
    a = np.asarray(a, np.float32)
    W1 = np.asarray(W1, np.float32)
    b1 = np.asarray(b1, np.float32)
    W2 = np.asarray(W2, np.float32)
    b2 = np.asarray(b2, np.float32)

    has_b2 = bool(np.any(b2))
    key = ("nc", has_b2)
    if key not in _cache:
        _cache[key] = _build(has_b2)
    nc = _cache[key]

    # xt[g]: [960, 4096] f16; xt[np*240+f, b*1024+nn*512+q] =
    #   x[4g+b, (2np+nn)*512+q, f]
    xg = []
    for g in range(2):
        arr = x[g * NB:(g + 1) * NB]            # [4, 4096, 240]
        arr = arr.transpose(2, 0, 1)            # [f, b, n]
        arr = arr.reshape(F, NB, 4, 2, 512)     # [f, b, np, nn, q]
        arr = arr.transpose(2, 0, 1, 3, 4)      # [np, f, b, nn, q]
        xg.append(np.ascontiguousarray(
            arr.reshape(4 * F, N)).astype(np.float16))

    # atp[j]: [512, 8192] f8e3; atp[(2c+t)*128+p, mb*512+q] =
    #   e3m4(a[j*1024 + c*512 + q, (16t+mb)*128+p] - 0.5)
    a8 = (a.T - np.float32(0.5)).astype(ml_dtypes.float8_e3m4)  # [m, n_out]
    aj = []
    for j in range(4):
        Aj = a8[:, j * NRC:(j + 1) * NRC]       # [4096 m, 1024 n]
        arr = Aj.reshape(2, 16, 128, 2, 512)    # [t, mb, p, c, q]
        arr = arr.transpose(3, 0, 2, 1, 4)      # [c, t, p, mb, q]
        aj.append(np.ascontiguousarray(arr.reshape(512, 8192)))

    w1p = np.zeros((F, 512), np.float16)
    for b in range(NB):
        w1p[:, 128 * b + 32 * b:128 * b + 32 * b + 32] = W1.astype(np.float16)
    # w2k[32b+h, hf*480 + b*120 + li] = W2[h, hf*120 + li]; zeros elsewhere
    w2k = np.zeros((128, 960), np.float16)
    for hf in range(2):
        for b in range(NB):
            w2k[32 * b:32 * b + 32, 480 * hf + 120 * b:480 * hf + 120 * b + 120] = \
                W2[:, 120 * hf:120 * hf + 120].astype(np.float16)
    b1s = np.ascontiguousarray(np.tile(b1, 4).reshape(128, 1))
    idnm = np.eye(128, dtype=np.float16)

    ins = []
    for c in range(8):
        g, j = c // 4, c % 4
        d = {"xt": xg[g], "atp": aj[j], "w1p": w1p, "w2k": w2k,
             "b1s": b1s, "idn": idnm}
        if has_b2:
            b2r = np.empty((1, 960), np.float16)
            for hf in range(2):
                for b in range(NB):
                    b2r[0, 480 * hf + 120 * b:480 * hf + 120 * b + 120] = \
                        b2[120 * hf:120 * hf + 120].astype(np.float16)
            d["b2r"] = b2r
        ins.append(d)

    trace = TRACE
    if trace:
        try:
            _install_ntff_hook()
        except Exception:
            trace = False
    r = run_bass_kernel_spmd(nc, ins, list(range(8)), trace=trace)
    last_exec_time_ns = r.exec_time_ns
    last_profile_json = r.profile_json

    res = np.empty((B, N, L), np.float32)
    for c in range(8):
        g, j = c // 4, c % 4
        # outp[p, (c2,s), hf, b, li]; n = (c2*4+s)*128 + p; l = hf*120+li
        arr = r.results[c]["outp"].reshape(128, 8, 2, NB, 120)
        res[g * NB:(g + 1) * NB, j * NRC:(j + 1) * NRC, :] = \
            arr.transpose(3, 1, 0, 2, 4).reshape(NB, NRC, L).astype(np.float32)
    return res
